# revision 10
# baseline (speedup 1.0000x reference)
"""Trainium2 Bass kernel for DenseRelativeLoc.

Computation (per batch b of 64):
  - gather 256 px-points and 256 py-points (columns of x[b] viewed as
    [C=768, HW=3136]) -> ptsT chunks [128c, 512s] via GPSIMD ap_gather
  - 3-layer MLP on the gathered features via TensorE matmuls in a
    transposed layout (activations kept as [feature-part, sample-free])
  - predxy [B*S, 2] written back; deltaxy computed host-side (pure
    integer arithmetic on the indices)

Sharding: data-parallel over batch, 8 batches per NeuronCore x 8 cores.
MLP weights replicated. No cross-core communication.

Pipeline notes: HWDGE rings effectively allow ~1 outstanding DMA per
issuing engine, and each DMA pays ~7us completion latency before its
consumer semaphore fires. So x is streamed as 16 half-batch loads
(4.7 MB each) alternating between the SP (nc.sync) and ACT (nc.scalar)
HWDGE rings, the 3 chunk-gathers of a half-batch are fused into one
ap_gather, and all constants ride in a single packed preload DMA.
ReLU+bias runs on DVE so the ACT ring stays DMA-only.
"""

import sys
import types
import contextlib
import ctypes

sys.path.insert(0, "/opt/trn_rl_repo")

import numpy as np

# ---------------------------------------------------------------- constants
B, C, H, W = 64, 768, 56, 56
HW = H * W            # 3136
S = 256               # points per batch (per side)
NIDX = 2 * S          # 512 gathered columns per batch (px then py)
NH = 256              # hidden width
OUT = 2
NCORES = 8
NB = B // NCORES      # batches per core = 8
KC = C // 128         # channel chunks = 6
HALF = 3              # chunks per half-batch load
GIDX = HALF * NIDX    # fused gather indices per half-batch = 1536

# packed const tensor column offsets (f32 elements per partition)
O_W1 = 0                       # [128, 12, 256] -> 3072
O_W2 = O_W1 + 2 * KC * NH      # [128, 2, 256] -> 512
O_W3 = O_W2 + 2 * NH           # [128, 2, 2] -> 4
O_B1 = O_W3 + 2 * OUT          # [128, 2]
O_B2 = O_B1 + 2
O_B3 = O_B2 + 2
NCONST = O_B3 + OUT

_PROGRAMS = {}        # cached compiled programs keyed by nb


def _install_ntff_hook():
    """Recreate antenv.axon_hooks (absent in this image) so that
    run_bass_kernel_spmd(trace=True) can register NTFF profiling."""
    import antenv

    if "antenv.axon_hooks" in sys.modules:
        return
    mod = types.ModuleType("antenv.axon_hooks")
    holder = {"hook": None}
    mod.set_axon_ntff_profile_hook = lambda h: holder.__setitem__("hook", h)
    mod.get_axon_ntff_profile_hook = lambda: holder["hook"]
    sys.modules["antenv.axon_hooks"] = mod
    antenv.axon_hooks = mod

    try:
        lib = ctypes.CDLL("/opt/axon/libaxon_pjrt.so")
    except OSError:
        return
    if not hasattr(lib, "axon_start_nrt_profile"):
        return
    lib.axon_start_nrt_profile.argtypes = [ctypes.POINTER(ctypes.c_int64), ctypes.c_size_t]
    lib.axon_start_nrt_profile.restype = ctypes.c_int64
    lib.axon_stop_nrt_profile.argtypes = [ctypes.c_char_p]
    lib.axon_stop_nrt_profile.restype = ctypes.c_int64

    @contextlib.contextmanager
    def _hook(output_dir, device_ids):
        import jax

        jax.devices()
        if device_ids:
            ids = (ctypes.c_int64 * len(device_ids))(*device_ids)
            rc = lib.axon_start_nrt_profile(ids, len(device_ids))
        else:
            rc = lib.axon_start_nrt_profile(None, 0)
        if rc != 0:
            raise RuntimeError(f"axon_start_nrt_profile rc={rc}")
        try:
            yield
        finally:
            n = lib.axon_stop_nrt_profile(str(output_dir).encode())
            print(f"profile: {n} file(s) written to {output_dir}", file=sys.stderr)

    mod.set_axon_ntff_profile_hook(_hook)


def build_program(nb=NB):
    """Build + compile the per-core Bass/Tile program (cached)."""
    if nb in _PROGRAMS:
        return _PROGRAMS[nb]

    import concourse.mybir as mybir
    import concourse.tile as tile
    from concourse import bacc
    from concourse.bass import ts, ds

    f32 = mybir.dt.float32
    i16 = mybir.dt.int16
    ADD = mybir.AluOpType.add
    MAX = mybir.AluOpType.max

    nc = bacc.Bacc("TRN2", target_bir_lowering=False, debug=False, num_devices=NCORES)

    # x is passed partition-major ([128, nb, KC, HW]) so each half-batch
    # load is one 37.6KB-contiguous HBM run per partition -> few, large
    # DMA descriptors (HWDGE descgen is ~100ns/descriptor and otherwise
    # caps effective bandwidth at ~120 GB/s)
    x_d = nc.dram_tensor("x", [128, nb, KC, HW], f32, kind="ExternalInput")
    idx_d = nc.dram_tensor("idx", [128, nb, GIDX // 16], i16, kind="ExternalInput")
    cst_d = nc.dram_tensor("cst", [128, NCONST], f32, kind="ExternalInput")
    pred_d = nc.dram_tensor("pred", [128, nb, 2, OUT], f32, kind="ExternalOutput")

    with tile.TileContext(nc) as tc:
        with (
            tc.tile_pool(name="xp", bufs=3) as xp,
            tc.tile_pool(name="gp", bufs=3) as gp,
            tc.tile_pool(name="wp", bufs=1) as wp,
            tc.tile_pool(name="hp", bufs=2) as hp,
            tc.tile_pool(name="op", bufs=1) as op,
            tc.tile_pool(name="ps1", bufs=2, space="PSUM") as ps1,
            tc.tile_pool(name="ps2", bufs=2, space="PSUM") as ps2,
            tc.tile_pool(name="ps3", bufs=2, space="PSUM") as ps3,
        ):
            cst = wp.tile([128, NCONST], f32, tag="cst")
            idxt = wp.tile([128, nb, GIDX // 16], i16, tag="idx")
            predt = op.tile([128, nb, 2, OUT], f32, tag="pred")

            nc.sync.dma_start(cst[:], cst_d.ap())
            nc.scalar.dma_start(idxt[:], idx_d.ap())

            def w1ap(j, nh):  # lhsT [128c, 128n] for W1 chunk j, n-half nh
                return cst[:, ds(O_W1 + j * NH + nh * 128, 128)]

            def w2ap(nk, mh):
                return cst[:, ds(O_W2 + nk * NH + mh * 128, 128)]

            def w3ap(mk):
                return cst[:, ds(O_W3 + mk * OUT, OUT)]

            for b in range(nb):
                # ---- stream x in two half-batch loads; fused gather per half
                gs = []
                for h in range(2):
                    xt = xp.tile([128, HALF, HW], f32, tag="x")
                    eng = nc.sync if h == 0 else nc.scalar
                    eng.dma_start(xt[:], x_d.ap()[:, b, ds(h * HALF, HALF), :])
                    g = gp.tile([128, GIDX], f32, tag="g")
                    nc.gpsimd.ap_gather(
                        g[:], xt[:], idxt[:, b, :],
                        channels=128, num_elems=HALF * HW, d=1, num_idxs=GIDX,
                    )
                    gs.append(g)

                # ---- layer 1: hdn1T[n, s] += W1-chunkT @ ptsT
                h1p = ps1.tile([128, 2, NH], f32, tag="h1p")
                for nh in range(2):
                    for k in range(KC):
                        hh, kk = divmod(k, HALF)
                        for xy in range(2):
                            nc.tensor.matmul(
                                h1p[:, nh, :],
                                w1ap(xy * KC + k, nh),
                                gs[hh][:, ds(kk * NIDX + xy * S, S)],
                                start=(k == 0 and xy == 0),
                                stop=(k == KC - 1 and xy == 1),
                            )
                h1 = hp.tile([128, 2, NH], f32, tag="h1")
                for nh in range(2):
                    nc.vector.tensor_scalar(
                        h1[:, nh, :], h1p[:, nh, :],
                        cst[:, ds(O_B1 + nh, 1)], 0.0, op0=ADD, op1=MAX,
                    )

                # ---- layer 2
                h2p = ps2.tile([128, 2, NH], f32, tag="h2p")
                for mh in range(2):
                    for nk in range(2):
                        nc.tensor.matmul(
                            h2p[:, mh, :],
                            w2ap(nk, mh),
                            h1[:, nk, :],
                            start=(nk == 0),
                            stop=(nk == 1),
                        )
                h2 = hp.tile([128, 2, NH], f32, tag="h2")
                for mh in range(2):
                    nc.vector.tensor_scalar(
                        h2[:, mh, :], h2p[:, mh, :],
                        cst[:, ds(O_B2 + mh, 1)], 0.0, op0=ADD, op1=MAX,
                    )

                # ---- layer 3 + b3
                pp = ps3.tile([128, 2, OUT], f32, tag="pp")
                for sh in range(2):
                    for mk in range(2):
                        nc.tensor.matmul(
                            pp[:, sh, :],
                            h2[:, mk, ts(sh, 128)],
                            w3ap(mk),
                            start=(mk == 0),
                            stop=(mk == 1),
                        )
                for sh in range(2):
                    nc.vector.tensor_scalar(
                        predt[:, b, sh, :], pp[:, sh, :],
                        cst[:, ds(O_B3, 1)], None, op0=ADD,
                    )

            nc.sync.dma_start(pred_d.ap(), predt[:])

    nc.compile()
    from concourse.bass_interp import get_hw_module

    nc.m = get_hw_module(nc.m)
    _PROGRAMS[nb] = nc
    return nc


def _prep_core_inputs(x, pxs, pys, W1, b1, W2, b2, W3, b3):
    """Host-side shard + layout massage. Returns list of 8 in_maps."""
    x = np.asarray(x, dtype=np.float32).reshape(B, C, HW)
    pxs = np.asarray(pxs).astype(np.int64)
    pys = np.asarray(pys).astype(np.int64)

    # flat gather indices for a fused 3-chunk gather: [B, 1536] int16,
    # wrapped [16, 96] and replicated to 128 partitions
    xi = pxs[:, :, 0] * H + pxs[:, :, 1]          # [B, S]
    yi = pys[:, :, 0] * H + pys[:, :, 1]          # [B, S]
    base = np.concatenate([xi, yi], axis=1)       # [B, 512]
    fused = np.concatenate([base + kk * HW for kk in range(HALF)], axis=1)  # [B, 1536]
    fused = fused.astype(np.int16)
    wrapped = fused.reshape(B, GIDX // 16, 16).transpose(0, 2, 1)   # [B, 16, 96]
    idx128 = np.tile(wrapped, (1, 8, 1)).reshape(B, 128, GIDX // 16)

    w1k = np.asarray(W1, dtype=np.float32).reshape(2 * KC, 128, NH).transpose(1, 0, 2)
    w2k = np.asarray(W2, dtype=np.float32).reshape(2, 128, NH).transpose(1, 0, 2)
    w3k = np.asarray(W3, dtype=np.float32).reshape(2, 128, OUT).transpose(1, 0, 2)
    b1t = np.asarray(b1, dtype=np.float32).reshape(2, 128).T
    b2t = np.asarray(b2, dtype=np.float32).reshape(2, 128).T
    b3t = np.broadcast_to(np.asarray(b3, dtype=np.float32), (128, OUT))

    cst = np.concatenate(
        [
            w1k.reshape(128, -1),
            w2k.reshape(128, -1),
            w3k.reshape(128, -1),
            b1t,
            b2t,
            b3t,
        ],
        axis=1,
    ).astype(np.float32)
    cst = np.ascontiguousarray(cst)
    assert cst.shape == (128, NCONST)

    in_maps = []
    for c in range(NCORES):
        sl = slice(c * NB, (c + 1) * NB)
        nb = sl.stop - sl.start
        xh = np.ascontiguousarray(
            x[sl].reshape(nb, KC, 128, HW).transpose(2, 0, 1, 3)
        )  # [128, nb, KC, HW], partition-major
        in_maps.append(
            {
                "x": xh,
                "idx": np.ascontiguousarray(idx128[sl].transpose(1, 0, 2)),
                "cst": cst,
            }
        )
    return in_maps


def _assemble_pred(results):
    """Per-core pred [128, NB, 2, 2] -> full predxy [B*S, 2]."""
    parts = []
    for c in range(NCORES):
        p = results[c]["pred"]  # [128, NB, 2, OUT]
        parts.append(np.ascontiguousarray(p.transpose(1, 2, 0, 3)).reshape(NB * 2 * 128, OUT))
    return np.concatenate(parts, axis=0)


def _run(inputs, trace=False):
    _install_ntff_hook()
    from concourse import bass_utils

    nc = build_program()
    in_maps = _prep_core_inputs(**inputs)
    res = bass_utils.run_bass_kernel_spmd(
        nc, in_maps, core_ids=list(range(NCORES)), trace=trace
    )
    predxy = _assemble_pred(res.results)

    pxs = np.asarray(inputs["pxs"]).astype(np.int64)
    pys = np.asarray(inputs["pys"]).astype(np.int64)
    deltaxy = (pxs - pys).astype(np.float32).reshape(-1, 2) + np.float32(H - 1)
    return (predxy, deltaxy), res


def kernel(**inputs):
    outs, _ = _run(inputs, trace=False)
    return outs


# revision 15
# speedup vs baseline: 1.0205x; 1.0205x over previous
"""Trainium2 Bass kernel for DenseRelativeLoc.

Computation (per batch b of 64):
  - gather 256 px-points and 256 py-points (columns of x[b] viewed as
    [C=768, HW=3136]) -> ptsT chunks [128c, 512s] via GPSIMD ap_gather
  - 3-layer MLP on the gathered features via TensorE matmuls in a
    transposed layout (activations kept as [feature-part, sample-free])
  - predxy [B*S, 2] written back; deltaxy computed host-side (pure
    integer arithmetic on the indices)

Sharding: data-parallel over batch, 8 batches per NeuronCore x 8 cores.
MLP weights replicated. No cross-core communication.

Pipeline notes: HWDGE rings effectively allow ~1 outstanding DMA per
issuing engine, and each DMA pays ~7us completion latency before its
consumer semaphore fires. So x is streamed as 16 half-batch loads
(4.7 MB each) alternating between the SP (nc.sync) and ACT (nc.scalar)
HWDGE rings, the 3 chunk-gathers of a half-batch are fused into one
ap_gather, and all constants ride in a single packed preload DMA.
ReLU+bias runs on DVE so the ACT ring stays DMA-only.
"""

import sys
import types
import contextlib
import ctypes

sys.path.insert(0, "/opt/trn_rl_repo")

import numpy as np

# ---------------------------------------------------------------- constants
B, C, H, W = 64, 768, 56, 56
HW = H * W            # 3136
S = 256               # points per batch (per side)
NIDX = 2 * S          # 512 gathered columns per batch (px then py)
NH = 256              # hidden width
OUT = 2
NCORES = 8
NB = B // NCORES      # batches per core = 8
KC = C // 128         # channel chunks = 6
HALF = 3              # chunks per half-batch load
GIDX = HALF * NIDX    # fused gather indices per half-batch = 1536

# packed const tensor column offsets (f32 elements per partition)
O_W1 = 0                       # [128, 12, 256] -> 3072
O_W2 = O_W1 + 2 * KC * NH      # [128, 2, 256] -> 512
O_W3 = O_W2 + 2 * NH           # [128, 2, 2] -> 4
O_B1 = O_W3 + 2 * OUT          # [128, 2]
O_B2 = O_B1 + 2
O_B3 = O_B2 + 2
NCONST = O_B3 + OUT

_PROGRAMS = {}        # cached compiled programs keyed by nb


def _install_ntff_hook():
    """Recreate antenv.axon_hooks (absent in this image) so that
    run_bass_kernel_spmd(trace=True) can register NTFF profiling."""
    import antenv

    if "antenv.axon_hooks" in sys.modules:
        return
    mod = types.ModuleType("antenv.axon_hooks")
    holder = {"hook": None}
    mod.set_axon_ntff_profile_hook = lambda h: holder.__setitem__("hook", h)
    mod.get_axon_ntff_profile_hook = lambda: holder["hook"]
    sys.modules["antenv.axon_hooks"] = mod
    antenv.axon_hooks = mod

    try:
        lib = ctypes.CDLL("/opt/axon/libaxon_pjrt.so")
    except OSError:
        return
    if not hasattr(lib, "axon_start_nrt_profile"):
        return
    lib.axon_start_nrt_profile.argtypes = [ctypes.POINTER(ctypes.c_int64), ctypes.c_size_t]
    lib.axon_start_nrt_profile.restype = ctypes.c_int64
    lib.axon_stop_nrt_profile.argtypes = [ctypes.c_char_p]
    lib.axon_stop_nrt_profile.restype = ctypes.c_int64

    @contextlib.contextmanager
    def _hook(output_dir, device_ids):
        import jax

        jax.devices()
        if device_ids:
            ids = (ctypes.c_int64 * len(device_ids))(*device_ids)
            rc = lib.axon_start_nrt_profile(ids, len(device_ids))
        else:
            rc = lib.axon_start_nrt_profile(None, 0)
        if rc != 0:
            raise RuntimeError(f"axon_start_nrt_profile rc={rc}")
        try:
            yield
        finally:
            n = lib.axon_stop_nrt_profile(str(output_dir).encode())
            print(f"profile: {n} file(s) written to {output_dir}", file=sys.stderr)

    mod.set_axon_ntff_profile_hook(_hook)


def build_program(nb=NB):
    """Build + compile the per-core Bass/Tile program (cached)."""
    if nb in _PROGRAMS:
        return _PROGRAMS[nb]

    import concourse.mybir as mybir
    import concourse.tile as tile
    from concourse import bacc
    from concourse.bass import ts, ds

    f32 = mybir.dt.float32
    i16 = mybir.dt.int16
    ADD = mybir.AluOpType.add
    MAX = mybir.AluOpType.max

    nc = bacc.Bacc("TRN2", target_bir_lowering=False, debug=False, num_devices=NCORES)

    # x is passed partition-major ([128, nb, KC, HW]) so each half-batch
    # load is one 37.6KB-contiguous HBM run per partition -> few, large
    # DMA descriptors (HWDGE descgen is ~100ns/descriptor and otherwise
    # caps effective bandwidth at ~120 GB/s)
    x_d = nc.dram_tensor("x", [128, nb, KC, HW], f32, kind="ExternalInput")
    idx_d = nc.dram_tensor("idx", [128, nb, NIDX // 16], i16, kind="ExternalInput")
    cst_d = nc.dram_tensor("cst", [128, NCONST], f32, kind="ExternalInput")
    pred_d = nc.dram_tensor("pred", [128, nb, 2, OUT], f32, kind="ExternalOutput")

    with tile.TileContext(nc) as tc:
        with (
            tc.tile_pool(name="xp", bufs=8) as xp,
            tc.tile_pool(name="gp", bufs=12) as gp,
            tc.tile_pool(name="wp", bufs=1) as wp,
            tc.tile_pool(name="hp", bufs=2) as hp,
            tc.tile_pool(name="op", bufs=1) as op,
            tc.tile_pool(name="ps1a", bufs=2, space="PSUM") as ps1a,
            tc.tile_pool(name="ps1b", bufs=2, space="PSUM") as ps1b,
            tc.tile_pool(name="ps2", bufs=2, space="PSUM") as ps2,
            tc.tile_pool(name="ps3", bufs=2, space="PSUM") as ps3,
        ):
            cst = wp.tile([128, NCONST], f32, tag="cst")
            idxt = wp.tile([128, nb, NIDX // 16], i16, tag="idx")
            predt = op.tile([128, nb, 2, OUT], f32, tag="pred")

            nc.sync.dma_start(cst[:], cst_d.ap())
            nc.scalar.dma_start(idxt[:], idx_d.ap())

            def w1ap(j, nh):  # lhsT [128c, 128n] for W1 chunk j, n-half nh
                return cst[:, ds(O_W1 + j * NH + nh * 128, 128)]

            def w2ap(nk, mh):
                return cst[:, ds(O_W2 + nk * NH + mh * 128, 128)]

            def w3ap(mk):
                return cst[:, ds(O_W3 + mk * OUT, OUT)]

            for b in range(nb):
                # ---- layer 1, streamed per 128-channel chunk:
                # load chunk -> gather 512 columns -> 4 accumulating matmuls.
                # The two n-half accumulation groups live in separate PSUM
                # banks so their matmuls may interleave per chunk.
                h1pa = ps1a.tile([128, NH], f32, tag="h1pa")
                h1pb = ps1b.tile([128, NH], f32, tag="h1pb")
                h1p = [h1pa, h1pb]
                for k in range(KC):
                    xt = xp.tile([128, HW], f32, tag="x")
                    eng = nc.sync if k % 2 == 0 else nc.scalar
                    eng.dma_start(xt[:], x_d.ap()[:, b, k, :])
                    g = gp.tile([128, NIDX], f32, tag="g")
                    nc.gpsimd.ap_gather(
                        g[:], xt[:], idxt[:, b, :],
                        channels=128, num_elems=HW, d=1, num_idxs=NIDX,
                    )
                    for nh in range(2):
                        for xy in range(2):
                            nc.tensor.matmul(
                                h1p[nh][:],
                                w1ap(xy * KC + k, nh),
                                g[:, ts(xy, S)],
                                start=(k == 0 and xy == 0),
                                stop=(k == KC - 1 and xy == 1),
                            )
                h1 = hp.tile([128, 2, NH], f32, tag="h1")
                for nh in range(2):
                    nc.vector.tensor_scalar(
                        h1[:, nh, :], h1p[nh][:],
                        cst[:, ds(O_B1 + nh, 1)], 0.0, op0=ADD, op1=MAX,
                    )

                # ---- layer 2
                h2p = ps2.tile([128, 2, NH], f32, tag="h2p")
                for mh in range(2):
                    for nk in range(2):
                        nc.tensor.matmul(
                            h2p[:, mh, :],
                            w2ap(nk, mh),
                            h1[:, nk, :],
                            start=(nk == 0),
                            stop=(nk == 1),
                        )
                h2 = hp.tile([128, 2, NH], f32, tag="h2")
                for mh in range(2):
                    nc.vector.tensor_scalar(
                        h2[:, mh, :], h2p[:, mh, :],
                        cst[:, ds(O_B2 + mh, 1)], 0.0, op0=ADD, op1=MAX,
                    )

                # ---- layer 3 + b3
                pp = ps3.tile([128, 2, OUT], f32, tag="pp")
                for sh in range(2):
                    for mk in range(2):
                        nc.tensor.matmul(
                            pp[:, sh, :],
                            h2[:, mk, ts(sh, 128)],
                            w3ap(mk),
                            start=(mk == 0),
                            stop=(mk == 1),
                        )
                for sh in range(2):
                    nc.vector.tensor_scalar(
                        predt[:, b, sh, :], pp[:, sh, :],
                        cst[:, ds(O_B3, 1)], None, op0=ADD,
                    )

            nc.sync.dma_start(pred_d.ap(), predt[:])

    nc.compile()
    from concourse.bass_interp import get_hw_module

    nc.m = get_hw_module(nc.m)
    _PROGRAMS[nb] = nc
    return nc


def _prep_core_inputs(x, pxs, pys, W1, b1, W2, b2, W3, b3):
    """Host-side shard + layout massage. Returns list of 8 in_maps."""
    x = np.asarray(x, dtype=np.float32).reshape(B, C, HW)
    pxs = np.asarray(pxs).astype(np.int64)
    pys = np.asarray(pys).astype(np.int64)

    # flat gather indices: [B, 512] int16, wrapped [16, 32] and
    # replicated to 128 partitions (each GPSIMD core reads its own 16)
    xi = pxs[:, :, 0] * H + pxs[:, :, 1]          # [B, S]
    yi = pys[:, :, 0] * H + pys[:, :, 1]          # [B, S]
    base = np.concatenate([xi, yi], axis=1).astype(np.int16)        # [B, 512]
    wrapped = base.reshape(B, NIDX // 16, 16).transpose(0, 2, 1)    # [B, 16, 32]
    idx128 = np.tile(wrapped, (1, 8, 1)).reshape(B, 128, NIDX // 16)

    w1k = np.asarray(W1, dtype=np.float32).reshape(2 * KC, 128, NH).transpose(1, 0, 2)
    w2k = np.asarray(W2, dtype=np.float32).reshape(2, 128, NH).transpose(1, 0, 2)
    w3k = np.asarray(W3, dtype=np.float32).reshape(2, 128, OUT).transpose(1, 0, 2)
    b1t = np.asarray(b1, dtype=np.float32).reshape(2, 128).T
    b2t = np.asarray(b2, dtype=np.float32).reshape(2, 128).T
    b3t = np.broadcast_to(np.asarray(b3, dtype=np.float32), (128, OUT))

    cst = np.concatenate(
        [
            w1k.reshape(128, -1),
            w2k.reshape(128, -1),
            w3k.reshape(128, -1),
            b1t,
            b2t,
            b3t,
        ],
        axis=1,
    ).astype(np.float32)
    cst = np.ascontiguousarray(cst)
    assert cst.shape == (128, NCONST)

    in_maps = []
    for c in range(NCORES):
        sl = slice(c * NB, (c + 1) * NB)
        nb = sl.stop - sl.start
        xh = np.ascontiguousarray(
            x[sl].reshape(nb, KC, 128, HW).transpose(2, 0, 1, 3)
        )  # [128, nb, KC, HW], partition-major
        in_maps.append(
            {
                "x": xh,
                "idx": np.ascontiguousarray(idx128[sl].transpose(1, 0, 2)),
                "cst": cst,
            }
        )
    return in_maps


def _assemble_pred(results):
    """Per-core pred [128, NB, 2, 2] -> full predxy [B*S, 2]."""
    parts = []
    for c in range(NCORES):
        p = results[c]["pred"]  # [128, NB, 2, OUT]
        parts.append(np.ascontiguousarray(p.transpose(1, 2, 0, 3)).reshape(NB * 2 * 128, OUT))
    return np.concatenate(parts, axis=0)


def _run(inputs, trace=False):
    _install_ntff_hook()
    from concourse import bass_utils

    nc = build_program()
    in_maps = _prep_core_inputs(**inputs)
    res = bass_utils.run_bass_kernel_spmd(
        nc, in_maps, core_ids=list(range(NCORES)), trace=trace
    )
    predxy = _assemble_pred(res.results)

    pxs = np.asarray(inputs["pxs"]).astype(np.int64)
    pys = np.asarray(inputs["pys"]).astype(np.int64)
    deltaxy = (pxs - pys).astype(np.float32).reshape(-1, 2) + np.float32(H - 1)
    return (predxy, deltaxy), res


def kernel(**inputs):
    outs, _ = _run(inputs, trace=False)
    return outs
